# revision 7
# baseline (speedup 1.0000x reference)
"""DNM_Linear Trainium2 kernel.

Computes, for x:[B,IN] f32, DNM_W:[OUT,M,IN] f32, q:[OUT,M,IN] f32 (constant qs):
    syn  = relu(K*(x[:,None,None,:]*DNM_W - q))      # [B,OUT,M,IN]
    soma = syn.sum(-1).sum(-1)                        # [B,OUT]
    out  = relu(K*(soma - QS))                        # [B,OUT]
with K=0.5, QS=0.1.

Strategy (8 NeuronCores, data-parallel over batch, 16 batches/core):
  * Identity: relu(z - c) = max(z, c) - c.  So per element we need only
    max(K*x*w, K*qs), and the "- K*qs" correction is a constant folded into
    the final affine (it sums to K*qs*M*IN per output neuron).
  * Layout: partitions = input dim i (4 tiles of 128), free = om = m*OUT + o
    (m-major, 4096 wide). x enters as the per-partition scalar operand of a
    single DVE tensor_scalar (op0=mult, op1=max) per (batch, i-tile) - fp16,
    SBUF->SBUF, unit stride => 4x perf mode. Optionally some i-tiles go to
    the scalar engine as activation(Relu, scale=K*x, bias=-K*qs) instead.
  * Reduction over i (the partition axis) via TensorE: matmul with a one-hot
    stationary [128 x 16] whose single ones-column routes each batch's
    column-sums into PSUM row b; all 64 (b,itile) x 8 om-chunks accumulate
    into 8 PSUM banks holding S[16b, 4096om].
  * m-sum: since om is m-major, summing the 16 m-blocks is 4 halving tree
    adds; final out = max(K*T, cf) - cf with cf = K*(qs*nmax*128*16 + QS).

kernel(**inputs) takes FULL inputs and returns the FULL [128,256] f32 output.
"""

import numpy as np

from concourse import bacc, bass, mybir, tile
from concourse.bass_utils import run_bass_kernel_spmd

B, IN, OUT, M = 128, 512, 256, 16
K, QS = 0.5, 0.1
NCORES = 8
BPC = B // NCORES          # batches per core
OM = OUT * M               # 4096, m-major: om = m*OUT + o
ITILES = IN // 128         # 4
NBANKS = OM // 512         # 8 psum banks
F16 = mybir.dt.float16
F32 = mybir.dt.float32

# i-tiles handled on the scalar engine (relu-form); rest on DVE (max-form).
ACT_ITILES = ()

_cache = {}


def _build_program(qs: float, act_itiles=ACT_ITILES):
    nc = bacc.Bacc("TRN2", target_bir_lowering=False)
    wt_d = nc.dram_tensor("wt", [ITILES, 128, OM], F16, kind="ExternalInput")
    xs_d = nc.dram_tensor("xs", [ITILES, 128, BPC], F32, kind="ExternalInput")
    out_d = nc.dram_tensor("out", [BPC, OUT], F32, kind="ExternalOutput")

    n_max_tiles = ITILES - len(act_itiles)       # i-tiles using max-form
    # max-form adds K*qs per synapse element; 128 i's per tile, 16 m's per o,
    # and the final activation scales by K again.
    cf = K * K * qs * n_max_tiles * 128 * M + K * QS

    mult = mybir.AluOpType.mult
    amax = mybir.AluOpType.max
    sub = mybir.AluOpType.subtract
    add = mybir.AluOpType.add
    relu = mybir.ActivationFunctionType.Relu

    with tile.TileContext(nc) as tc:
        with (
            tc.tile_pool(name="const", bufs=1) as cpool,
            tc.tile_pool(name="work", bufs=4) as work,
            tc.tile_pool(name="tail", bufs=1) as tail,
            tc.tile_pool(name="psum", bufs=1, space="PSUM") as pp,
        ):
            wt = [
                cpool.tile([128, OM], F16, name=f"wt{t}", tag=f"wt{t}")
                for t in range(ITILES)
            ]
            xs = cpool.tile([128, ITILES, BPC], F32)
            oh = cpool.tile([128, BPC * BPC], F16)   # 16 one-hot matrices
            for t in range(ITILES):
                nc.sync.dma_start(wt[t][:, :], wt_d[t])
            nc.sync.dma_start(xs[:, :, :], xs_d.rearrange("t p b -> p t b"))
            nc.vector.memset(oh[:, :], 0.0)
            for b in range(BPC):
                nc.vector.memset(oh[:, b * BPC + b : b * BPC + b + 1], 1.0)

            psum = [
                pp.tile([BPC, 512], F32, name=f"ps{c}", tag=f"ps{c}")
                for c in range(NBANKS)
            ]

            first = True
            for b in range(BPC):
                for t in range(ITILES):
                    u = work.tile([128, OM], F16, tag="u")
                    if t in act_itiles:
                        nc.scalar.activation(
                            u[:, :], wt[t][:, :], relu,
                            bias=-K * qs, scale=xs[:, t, b : b + 1],
                        )
                    else:
                        nc.vector.tensor_scalar(
                            u[:, :], wt[t][:, :],
                            xs[:, t, b : b + 1], K * qs, mult, amax,
                        )
                    last = b == BPC - 1 and t == ITILES - 1
                    for c in range(NBANKS):
                        nc.tensor.matmul(
                            psum[c][:, :],
                            oh[:, b * BPC : b * BPC + BPC],
                            u[:, 512 * c : 512 * (c + 1)],
                            start=first, stop=last,
                        )
                    first = False

            # Tail: drain PSUM -> S[16, 4096], sum the 16 m-major blocks,
            # then out = max(K*T, cf) - cf.
            S = tail.tile([BPC, OM], F32)
            for c in range(NBANKS):
                nc.scalar.copy(S[:, 512 * c : 512 * (c + 1)], psum[c][:, :])
            t1 = tail.tile([BPC, OM // 2], F32)
            nc.vector.tensor_tensor(t1[:, :], S[:, : OM // 2], S[:, OM // 2 :], add)
            t2 = tail.tile([BPC, OM // 4], F32)
            nc.vector.tensor_tensor(t2[:, :], t1[:, : OM // 4], t1[:, OM // 4 :], add)
            t3 = tail.tile([BPC, OM // 8], F32)
            nc.vector.tensor_tensor(t3[:, :], t2[:, : OM // 8], t2[:, OM // 8 :], add)
            t4 = tail.tile([BPC, OUT], F32)
            nc.vector.tensor_tensor(t4[:, :], t3[:, :OUT], t3[:, OUT:], add)
            f1 = tail.tile([BPC, OUT], F32)
            nc.vector.tensor_scalar(f1[:, :], t4[:, :], K, cf, mult, amax)
            fo = tail.tile([BPC, OUT], F32)
            nc.vector.tensor_scalar(fo[:, :], f1[:, :], cf, None, sub)
            nc.sync.dma_start(out_d[:, :], fo[:, :])

    nc.compile()
    return nc


def _prep_inputs(x, DNM_W):
    # WT[i, om] with om = m*OUT + o  (m-major so the m-sum is 4 tree adds)
    wmo = np.ascontiguousarray(
        np.asarray(DNM_W, np.float32).transpose(1, 0, 2)
    ).reshape(OM, IN)
    wt = np.ascontiguousarray(wmo.T).astype(np.float16).reshape(ITILES, 128, OM)
    # xs[t, i, b] = K * x[b, t*128 + i]
    xs = (K * np.asarray(x, np.float32).T).reshape(ITILES, 128, B)
    return wt, xs


def _run(x, DNM_W, qs, trace=False):
    key = (qs, ACT_ITILES)
    if key not in _cache:
        _cache[key] = _build_program(qs)
    nc = _cache[key]
    wt, xs = _prep_inputs(x, DNM_W)
    in_maps = [
        {"wt": wt, "xs": np.ascontiguousarray(xs[:, :, i * BPC : (i + 1) * BPC])}
        for i in range(NCORES)
    ]
    res = run_bass_kernel_spmd(nc, in_maps, list(range(NCORES)), trace=trace)
    out = np.concatenate([res.results[i]["out"] for i in range(NCORES)], axis=0)
    return out.astype(np.float32), res


def kernel(x, DNM_W, q):
    q = np.asarray(q, np.float32)
    qs = float(q.reshape(-1)[0])
    if not np.all(q == qs):
        # General-q fallback (never hit for this problem's setup: q is
        # init.constant_): exact reference math on host.
        x32 = np.asarray(x, np.float32)
        w32 = np.asarray(DNM_W, np.float32)
        soma = np.zeros((B, OUT), np.float32)
        for o in range(OUT):
            syn = np.maximum(K * (x32[:, None, :] * w32[o] - q[o]), 0.0)
            soma[:, o] = syn.sum(axis=(1, 2))
        return np.maximum(K * (soma - QS), 0.0).astype(np.float32)
    out, _ = _run(x, DNM_W, qs)
    return out
